# revision 11
# baseline (speedup 1.0000x reference)
"""Trainium2 Bass kernel for nn_CachedShapingFunctions (embedding_lookup).

out[b,t,w] = interp of lookup_table[:, w] at uniform-bucket position of
inputs[b,t,w].  Data-parallel over batch across 8 NeuronCores; the LUT is
replicated as a host-prepared bf16 (value, delta) pair table per waveshaper.

Per-core pipeline (shard flattened to [65536, 64], 64 super-tiles of
[128 part = 2 time-chunks x 64 w, 512 t]):
  - DMA in natural tiles, PE-transpose to waveshaper-on-partition layout
  - DVE: 7-op index pipeline (clamp, round(ic-0.5) floor trick, fraction)
  - GPSIMD ap_gather of bf16 (value, delta) pairs (the dominant cost,
    ~372us/tile -- command-bound at ~45ns/idx)
  - strided-partition extraction of the 1/16-dense gather output
    (split ACT/DVE), interpolation, PE-transpose back, DMA out

Scheduling: ap_gather contends catastrophically with concurrent DVE work
(shared POOL/DVE SBUF port), so all DVE/ACT work is packed into a serial
window between gathers via buffer reuse (bufs=1 pools) and a dummy
dependency-chaining op.  DMA and PE overlap the gather freely.
"""
import sys
import numpy as np

sys.path.insert(0, '/opt/trn_rl_repo')

import bass_rust
import concourse.bass as bass
import concourse.mybir as mybir
import concourse.tile as tile_mod
from concourse.tile import TileContext
from concourse.vector_clock import ScopedClock
from concourse import library_config

MIN_VALUE, MAX_VALUE = -3.0, 3.0
NB = 4096          # buckets
W = 64             # waveshapers
N_CORES = 8

# ---------------------------------------------------------------- patches --
# This walrus build accepts at most ONE sync-wait per instruction.  The Tile
# tail drain and scheduler can attach more; spill the excess onto nops.

_MAXW = 1

def _spill_waits(nc):
    for f in nc.m.functions:
        for bb in f.blocks:
            out = []
            for inst in list(bb.instructions):
                si = inst.sync_info
                if si is not None and len(si.on_wait) > _MAXW:
                    waits = list(si.on_wait)
                    spill = waits[:-_MAXW]
                    for i in range(0, len(spill), _MAXW):
                        nop = mybir.InstNoOp(
                            name=f"wspill_{inst.name}_{i}", ins=[], outs=[])
                        nop.engine = inst.engine
                        nop.sync_info = bass_rust.SyncInfo(
                            on_wait=spill[i:i + _MAXW], on_update=[])
                        out.append(nop)
                    inst.sync_info = bass_rust.SyncInfo(
                        on_wait=waits[-_MAXW:], on_update=list(si.on_update))
                out.append(inst)
            bb.instructions = out


def _patched_drain_and_barrier(self, tick_clock, wait_clock):
    nc = self.nc
    drain_inst = nc.sync.drain()
    wait_clock.add_sem_waits(
        drain_inst.ins, ScopedClock({None: tick_clock.global_clock}))
    si = drain_inst.ins.sync_info
    if si is not None and len(si.on_wait) > _MAXW:
        waits = list(si.on_wait)
        drain_inst.ins.sync_info = bass_rust.SyncInfo(
            on_wait=waits[:_MAXW], on_update=list(si.on_update))
        rest = waits[_MAXW:]
        for i in range(0, len(rest), _MAXW):
            nop = nc.sync.nop(hint="drain_wait_spill", nofuse=True)
            nop.ins.sync_info = bass_rust.SyncInfo(
                on_wait=rest[i:i + _MAXW], on_update=[])
    nc.all_engine_barrier()
    assert self.sems is not None
    popped = nc._tile_sem_poison_stack.pop()
    assert popped is self._sem_poison
    nc.clear_and_free_semaphores(list(self.sems.allocated().values()))
    nc.all_engine_barrier()


tile_mod.TileContext._drain_and_barrier = _patched_drain_and_barrier

# ----------------------------------------------------------------- kernel --

S = 512            # t-columns per transposed super-tile (per chunk)
TROWS = 2 * S      # natural t rows covered per super-tile (2 chunks)

F32 = mybir.dt.float32
I16 = mybir.dt.int16
BF16 = mybir.dt.bfloat16


def build_kernel(n_rows):
    """n_rows: flattened time rows per core (65536 full scale)."""
    assert n_rows % TROWS == 0
    n_tiles = n_rows // TROWS
    nc = bass.Bass()
    x_d = nc.dram_tensor("x", [n_rows, W], F32, kind="ExternalInput")
    tbl_d = nc.dram_tensor("tbl", [128, NB * 2], BF16, kind="ExternalInput")
    id_d = nc.dram_tensor("ident", [128, 128], F32, kind="ExternalInput")
    mk_d = nc.dram_tensor("masks", [128, 16], mybir.dt.int32, kind="ExternalInput")
    y_d = nc.dram_tensor("y", [n_rows, W], F32, kind="ExternalOutput")

    A = mybir.AluOpType

    with TileContext(nc) as tc:
        with (
            tc.tile_pool(name="const", bufs=1) as cpool,
            tc.tile_pool(name="io", bufs=3) as iop,
            tc.tile_pool(name="xt", bufs=2) as xtp,
            tc.tile_pool(name="sc", bufs=1) as scp,
            tc.tile_pool(name="sp", bufs=1) as spp,
            tc.tile_pool(name="on", bufs=2) as onp,
            tc.tile_pool(name="psi", bufs=4, space="PSUM") as psip,
            tc.tile_pool(name="pso", bufs=2, space="PSUM") as psop,
        ):
            tbl = cpool.tile([128, NB * 2], BF16)
            ident = cpool.tile([128, 128], F32)
            masks = cpool.tile([128, 16], mybir.dt.int32)
            nc.sync.dma_start(tbl[:, :], tbl_d[:, :])
            nc.sync.dma_start(ident[:, :], id_d[:, :])
            nc.sync.dma_start(masks[:, :], mk_d[:, :])
            nc.gpsimd.load_library(library_config.ap_gather)
            tbl3 = tbl[:, :].rearrange("p (n d) -> p n d", d=2)

            xnats = {}

            def emit_dma_in(i):
                xnat = iop.tile([128, 8 * W], F32, tag="xnat")
                in_ap = bass.AP(x_d, i * TROWS * W, [[W, 128], [128 * W, 8], [1, W]])
                nc.sync.dma_start(
                    xnat[:, :].rearrange("p (s w) -> p s w", s=8), in_ap)
                xnats[i] = xnat

            def emit_intrans_pe(i):
                """PE transposes for tile i (runs free, overlaps gather)."""
                xnat = xnats.pop(i)
                psts = []
                for k in range(4):
                    pst = psip.tile([128, 128], F32, tag="psin")
                    nc.tensor.transpose(
                        pst[:, :], xnat[:, 128 * k: 128 * k + 128], ident)
                    psts.append(pst)
                return psts

            def emit_intrans_act(i, psts):
                """PSUM -> xT copies (ACT, in the serial window)."""
                xT = xtp.tile([128, S], F32, tag="xT")
                for k in range(4):
                    nc.scalar.copy(xT[:, 128 * k: 128 * k + 128], psts[k][:, :])
                return xT

            def emit_idxprep(i, xT):
                """DVE index pipeline into an i16 scratch (ii) + fraction ff.
                The real idx buffer is written separately (emit_idx_write) so
                the dummy chain op can be ordered before it."""
                ic = scp.tile([128, S], F32, tag="ic")
                icc = scp.tile([128, S], F32, tag="icc")
                tmp = scp.tile([128, S], F32, tag="tmp")
                ii = scp.tile([128, S], I16, tag="ii")
                ilf = scp.tile([128, S], F32, tag="ilf")
                icm = scp.tile([128, S], F32, tag="icm")
                ff = scp.tile([128, S], F32, tag="ff")
                nc.vector.tensor_scalar(ic[:, :], xT[:, :], 3.0, 682.5, A.add, A.mult)
                nc.vector.tensor_scalar(icc[:, :], ic[:, :], 0.0, 4095.0, A.max, A.min)
                # floor(icc) via round-to-nearest(icc - (0.5 - eps)); off-by-one
                # at segment boundaries is harmless (interpolation continuity).
                nc.vector.tensor_scalar(tmp[:, :], icc[:, :], 0.49999997, None, A.subtract)
                nc.vector.tensor_copy(ii[:, :], tmp[:, :])           # f32 -> i16 RNE
                nc.vector.tensor_copy(ilf[:, :], ii[:, :])           # i16 -> f32
                # upper-clamped ic for the fraction: keeps lower extrapolation
                # exact and forces out = T[4095] for ic >= 4095.
                nc.vector.tensor_scalar(icm[:, :], ic[:, :], 4095.0, None, A.min)
                nc.vector.tensor_tensor(ff[:, :], icm[:, :], ilf[:, :], A.subtract)
                return ii, ff

            def emit_idx_write(idx, ii):
                nc.vector.tensor_copy(idx[:, :], ii[:, :])           # i16 -> i16

            def emit_gather(i, idx):
                sparse = spp.tile([128, 16 * S * 2], BF16, tag="sparse")
                sp3 = sparse[:, :].rearrange("p (n d) -> p n d", d=2)
                nc.gpsimd.ap_gather(sp3, tbl3, idx[:, :], channels=128,
                                    num_elems=NB, d=2, num_idxs=16 * S)
                return sparse

            def emit_extract_interp(i, sparse, ff, masks):
                """Compact the 1/16-dense gather output: partition p keeps
                pair slots m = 16n + (p%16).  Done as 16 bitwise masked
                merges in the int32 (pair = one u32) domain, then
                interpolate from the bf16 view."""
                pairs = scp.tile([128, S], mybir.dt.int32, tag="pairs")
                spi = sparse[:, :].bitcast(mybir.dt.int32)
                sp16 = spi.rearrange("p (n q) -> p n q", q=16)
                for r in range(16):
                    mcol = masks[:, r: r + 1]
                    if r == 0:
                        nc.vector.tensor_scalar(
                            pairs[:, :], sp16[:, :, 0], mcol, None, A.bitwise_and)
                    else:
                        nc.vector.scalar_tensor_tensor(
                            pairs[:, :], sp16[:, :, r], mcol, pairs[:, :],
                            A.bitwise_and, A.bitwise_or)
                pr3 = pairs[:, :].bitcast(BF16).rearrange("p (n q) -> p n q", q=2)
                outT = scp.tile([128, S], F32, tag="outT")
                nc.vector.tensor_tensor(outT[:, :], ff[:, :], pr3[:, :, 1], A.mult)
                nc.vector.tensor_tensor(outT[:, :], outT[:, :], pr3[:, :, 0], A.add)
                return outT

            def emit_out(i, outT):
                onat = onp.tile([128, 8 * W], F32, tag="onat")
                for k in range(4):
                    pst = psop.tile([128, 128], F32, tag="psout")
                    nc.tensor.transpose(
                        pst[:, :], outT[:, 128 * k: 128 * k + 128], ident)
                    nc.scalar.copy(onat[:, 128 * k: 128 * k + 128], pst[:, :])
                out_ap = bass.AP(y_d, i * TROWS * W, [[W, 128], [128 * W, 8], [1, W]])
                nc.sync.dma_start(
                    out_ap, onat[:, :].rearrange("p (s w) -> p s w", s=8))
                return onat

            # idx is a single buffer reused across tiles (bufs=1 semantics):
            # gather(i+1) waits on its last writer, which is emit_idx_write
            # for tile i+1 -- ordered on the DVE queue after the dummy chain
            # op, which itself waits on tile i's output copies.  This pins
            # all DVE/ACT work into the window between gathers.
            idx = scp.tile([128, S], I16, tag="idx")

            # -------- warmup
            emit_dma_in(0)
            emit_dma_in(1)
            psts0 = emit_intrans_pe(0)
            xT0 = emit_intrans_act(0, psts0)
            ii_next, ff_next = emit_idxprep(0, xT0)
            emit_idx_write(idx, ii_next)

            # -------- main loop
            for i in range(n_tiles):
                ff = ff_next
                sparse = emit_gather(i, idx)                  # phase A
                if i + 2 < n_tiles:
                    emit_dma_in(i + 2)                        # overlaps A
                psts = emit_intrans_pe(i + 1) if i + 1 < n_tiles else None

                # phase B (serial window, opens when gather i completes):
                outT = emit_extract_interp(i, sparse, ff, masks)
                onat = emit_out(i, outT)
                if i + 1 < n_tiles:
                    xT = emit_intrans_act(i + 1, psts)
                    ii_next, ff_next = emit_idxprep(i + 1, xT)
                    # dummy chain: a DVE op that reads onat(i) (so it waits
                    # for the ACT output copies of tile i) and writes idx
                    # col 0; the real idx write follows it on the DVE queue
                    # and overwrites all columns.  Net effect: gather(i+1)
                    # cannot start before tile i's output path is done.
                    nc.vector.tensor_copy(idx[:, 0:1], onat[:, 0:1])
                    emit_idx_write(idx, ii_next)

    from concourse.library_overlay import lower_extended_insts
    lower_extended_insts(nc)
    _spill_waits(nc)
    return nc


def make_table(lookup_table):
    import ml_dtypes
    lut = np.asarray(lookup_table, dtype=np.float32)          # [4096, 64]
    vu = np.concatenate([lut[1:], lut[-1:]], axis=0)          # T[min(i+1,4095)]
    delta = vu - lut                                          # f32 exact
    val_bf = lut.astype(ml_dtypes.bfloat16)
    delta_bf = delta.astype(ml_dtypes.bfloat16)
    pair = np.stack([val_bf, delta_bf], axis=-1)              # [4096, 64, 2]
    tblw = np.ascontiguousarray(pair.transpose(1, 0, 2)).reshape(W, NB * 2)
    tbl128 = np.concatenate([tblw, tblw], axis=0)             # [128, 8192]
    return np.ascontiguousarray(tbl128)


def make_masks():
    p = np.arange(128)
    m = np.where(p[:, None] % 16 == np.arange(16)[None, :], -1, 0)
    return m.astype(np.int32)                                 # [128, 16]


_CACHE = {}


def kernel(inputs, lookup_table):
    x = np.ascontiguousarray(np.asarray(inputs, dtype=np.float32))
    B, T, Wx = x.shape
    assert Wx == W
    per_core_b = B // N_CORES
    n_rows = per_core_b * T
    tbl = make_table(lookup_table)
    ident = np.eye(128, dtype=np.float32)
    masks = make_masks()

    if n_rows not in _CACHE:
        _CACHE[n_rows] = build_kernel(n_rows)
    nc = _CACHE[n_rows]

    from concourse import bass_utils
    shards = x.reshape(N_CORES, n_rows, W)
    in_maps = [{"x": shards[c], "tbl": tbl, "ident": ident, "masks": masks}
               for c in range(N_CORES)]
    res = bass_utils.run_bass_kernel_spmd(
        nc, in_maps, core_ids=list(range(N_CORES)))
    out = np.stack([res.results[c]["y"] for c in range(N_CORES)], axis=0)
    return out.reshape(B, T, W)


# revision 14
# speedup vs baseline: 1.0630x; 1.0630x over previous
"""Trainium2 Bass kernel for nn_CachedShapingFunctions (embedding_lookup).

out[b,t,w] = interp of lookup_table[:, w] at uniform-bucket position of
inputs[b,t,w].  Data-parallel over batch across 8 NeuronCores; the LUT is
replicated as a host-prepared bf16 (value, delta) pair table per waveshaper.

Per-core pipeline (shard flattened to [65536, 64], 64 super-tiles of
[128 part = 2 time-chunks x 64 w, 512 t]):
  - DMA in natural tiles, PE-transpose to waveshaper-on-partition layout
  - DVE: 7-op index pipeline (clamp, round(ic-0.5) floor trick, fraction)
  - GPSIMD ap_gather of bf16 (value, delta) pairs (the dominant cost,
    ~372us/tile -- command-bound at ~45ns/idx)
  - strided-partition extraction of the 1/16-dense gather output
    (split ACT/DVE), interpolation, PE-transpose back, DMA out

Scheduling: ap_gather contends catastrophically with concurrent DVE work
(shared POOL/DVE SBUF port), so all DVE/ACT work is packed into a serial
window between gathers via buffer reuse (bufs=1 pools) and a dummy
dependency-chaining op.  DMA and PE overlap the gather freely.
"""
import sys
import numpy as np

sys.path.insert(0, '/opt/trn_rl_repo')

import bass_rust
import concourse.bass as bass
import concourse.mybir as mybir
import concourse.tile as tile_mod
from concourse.tile import TileContext
from concourse.vector_clock import ScopedClock
from concourse import library_config

MIN_VALUE, MAX_VALUE = -3.0, 3.0
NB = 4096          # buckets
W = 64             # waveshapers
N_CORES = 8

# ---------------------------------------------------------------- patches --
# This walrus build accepts at most ONE sync-wait per instruction.  The Tile
# tail drain and scheduler can attach more; spill the excess onto nops.

_MAXW = 1

def _spill_waits(nc):
    for f in nc.m.functions:
        for bb in f.blocks:
            out = []
            for inst in list(bb.instructions):
                si = inst.sync_info
                if si is not None and len(si.on_wait) > _MAXW:
                    waits = list(si.on_wait)
                    spill = waits[:-_MAXW]
                    for i in range(0, len(spill), _MAXW):
                        nop = mybir.InstNoOp(
                            name=f"wspill_{inst.name}_{i}", ins=[], outs=[])
                        nop.engine = inst.engine
                        nop.sync_info = bass_rust.SyncInfo(
                            on_wait=spill[i:i + _MAXW], on_update=[])
                        out.append(nop)
                    inst.sync_info = bass_rust.SyncInfo(
                        on_wait=waits[-_MAXW:], on_update=list(si.on_update))
                out.append(inst)
            bb.instructions = out


def _patched_drain_and_barrier(self, tick_clock, wait_clock):
    nc = self.nc
    drain_inst = nc.sync.drain()
    wait_clock.add_sem_waits(
        drain_inst.ins, ScopedClock({None: tick_clock.global_clock}))
    si = drain_inst.ins.sync_info
    if si is not None and len(si.on_wait) > _MAXW:
        waits = list(si.on_wait)
        drain_inst.ins.sync_info = bass_rust.SyncInfo(
            on_wait=waits[:_MAXW], on_update=list(si.on_update))
        rest = waits[_MAXW:]
        for i in range(0, len(rest), _MAXW):
            nop = nc.sync.nop(hint="drain_wait_spill", nofuse=True)
            nop.ins.sync_info = bass_rust.SyncInfo(
                on_wait=rest[i:i + _MAXW], on_update=[])
    nc.all_engine_barrier()
    assert self.sems is not None
    popped = nc._tile_sem_poison_stack.pop()
    assert popped is self._sem_poison
    nc.clear_and_free_semaphores(list(self.sems.allocated().values()))
    nc.all_engine_barrier()


tile_mod.TileContext._drain_and_barrier = _patched_drain_and_barrier

# ----------------------------------------------------------------- kernel --

S = 512            # t-columns per transposed super-tile (per chunk)
TROWS = 2 * S      # natural t rows covered per super-tile (2 chunks)

F32 = mybir.dt.float32
I16 = mybir.dt.int16
BF16 = mybir.dt.bfloat16


def build_kernel(n_rows):
    """n_rows: flattened time rows per core (65536 full scale)."""
    assert n_rows % TROWS == 0
    n_tiles = n_rows // TROWS
    nc = bass.Bass()
    x_d = nc.dram_tensor("x", [n_rows, W], F32, kind="ExternalInput")
    tbl_d = nc.dram_tensor("tbl", [128, NB * 2], BF16, kind="ExternalInput")
    id_d = nc.dram_tensor("ident", [128, 128], F32, kind="ExternalInput")
    mk_d = nc.dram_tensor("masks", [128, 16], mybir.dt.int32, kind="ExternalInput")
    y_d = nc.dram_tensor("y", [n_rows, W], F32, kind="ExternalOutput")

    A = mybir.AluOpType

    with TileContext(nc) as tc:
        with (
            tc.tile_pool(name="const", bufs=1) as cpool,
            tc.tile_pool(name="io", bufs=3) as iop,
            tc.tile_pool(name="xt", bufs=1) as xtp,
            tc.tile_pool(name="sc", bufs=1) as scp,
            tc.tile_pool(name="sp", bufs=1) as spp,
            tc.tile_pool(name="on", bufs=2) as onp,
            tc.tile_pool(name="psi", bufs=4, space="PSUM") as psip,
            tc.tile_pool(name="pso", bufs=2, space="PSUM") as psop,
        ):
            tbl = cpool.tile([128, NB * 2], BF16)
            ident = cpool.tile([128, 128], F32)
            masks = cpool.tile([128, 16], mybir.dt.int32)
            nc.sync.dma_start(tbl[:, :], tbl_d[:, :])
            nc.sync.dma_start(ident[:, :], id_d[:, :])
            nc.sync.dma_start(masks[:, :], mk_d[:, :])
            nc.gpsimd.load_library(library_config.ap_gather)
            tbl3 = tbl[:, :].rearrange("p (n d) -> p n d", d=2)

            xnats = {}

            def emit_dma_in(i):
                xnat = iop.tile([128, 8 * W], F32, tag="xnat")
                in_ap = bass.AP(x_d, i * TROWS * W, [[W, 128], [128 * W, 8], [1, W]])
                nc.sync.dma_start(
                    xnat[:, :].rearrange("p (s w) -> p s w", s=8), in_ap)
                xnats[i] = xnat

            def emit_intrans_pe(i):
                """PE transposes for tile i (runs free, overlaps gather)."""
                xnat = xnats.pop(i)
                psts = []
                for k in range(4):
                    pst = psip.tile([128, 128], F32, tag="psin")
                    nc.tensor.transpose(
                        pst[:, :], xnat[:, 128 * k: 128 * k + 128], ident)
                    psts.append(pst)
                return psts

            def emit_intrans_act(i, psts):
                """PSUM -> xT copies (ACT, in the serial window)."""
                xT = xtp.tile([128, S], F32, tag="xT")
                for k in range(4):
                    nc.scalar.copy(xT[:, 128 * k: 128 * k + 128], psts[k][:, :])
                return xT

            def emit_idxprep(i, xT, idx):
                """DVE index pipeline (7 ops): writes idx (i16) + fraction ff."""
                ic = scp.tile([128, S], F32, tag="ic")
                icc = scp.tile([128, S], F32, tag="icc")
                tmp = scp.tile([128, S], F32, tag="tmp")
                ilf = scp.tile([128, S], F32, tag="ilf")
                icm = scp.tile([128, S], F32, tag="icm")
                ff = scp.tile([128, S], F32, tag="ff")
                nc.vector.tensor_scalar(ic[:, :], xT[:, :], 3.0, 682.5, A.add, A.mult)
                nc.vector.tensor_scalar(icc[:, :], ic[:, :], 0.0, 4095.0, A.max, A.min)
                # floor(icc) via round-to-nearest(icc - (0.5 - eps)); off-by-one
                # at segment boundaries is harmless (interpolation continuity).
                nc.vector.tensor_scalar(tmp[:, :], icc[:, :], 0.49999997, None, A.subtract)
                nc.vector.tensor_copy(idx[:, :], tmp[:, :])          # f32 -> i16 RNE
                nc.vector.tensor_copy(ilf[:, :], idx[:, :])          # i16 -> f32
                # upper-clamped ic for the fraction: keeps lower extrapolation
                # exact and forces out = T[4095] for ic >= 4095.
                nc.vector.tensor_scalar(icm[:, :], ic[:, :], 4095.0, None, A.min)
                nc.vector.tensor_tensor(ff[:, :], icm[:, :], ilf[:, :], A.subtract)
                return ff

            def emit_gather(i, idx):
                sparse = spp.tile([128, 16 * S * 2], BF16, tag="sparse")
                sp3 = sparse[:, :].rearrange("p (n d) -> p n d", d=2)
                nc.gpsimd.ap_gather(sp3, tbl3, idx[:, :], channels=128,
                                    num_elems=NB, d=2, num_idxs=16 * S)
                return sparse

            def emit_extract_interp(i, sparse, ff, masks):
                """Compact the 1/16-dense gather output: partition p keeps
                pair slots m = 16n + (p%16).  Done as 16 bitwise masked
                merges in the int32 (pair = one u32) domain, then
                interpolate from the bf16 view."""
                pairs = scp.tile([128, S], mybir.dt.int32, tag="pairs")
                spi = sparse[:, :].bitcast(mybir.dt.int32)
                sp16 = spi.rearrange("p (n q) -> p n q", q=16)
                for r in range(16):
                    mcol = masks[:, r: r + 1]
                    if r == 0:
                        nc.vector.tensor_scalar(
                            pairs[:, :], sp16[:, :, 0], mcol, None, A.bitwise_and)
                    else:
                        nc.vector.scalar_tensor_tensor(
                            pairs[:, :], sp16[:, :, r], mcol, pairs[:, :],
                            A.bitwise_and, A.bitwise_or)
                pr3 = pairs[:, :].bitcast(BF16).rearrange("p (n q) -> p n q", q=2)
                outT = scp.tile([128, S], F32, tag="outT")
                nc.vector.tensor_tensor(outT[:, :], ff[:, :], pr3[:, :, 1], A.mult)
                nc.vector.tensor_tensor(outT[:, :], outT[:, :], pr3[:, :, 0], A.add)
                return outT

            def emit_out(i, outT):
                onat = onp.tile([128, 8 * W], F32, tag="onat")
                for k in range(4):
                    pst = psop.tile([128, 128], F32, tag="psout")
                    nc.tensor.transpose(
                        pst[:, :], outT[:, 128 * k: 128 * k + 128], ident)
                    nc.scalar.copy(onat[:, 128 * k: 128 * k + 128], pst[:, :])
                out_ap = bass.AP(y_d, i * TROWS * W, [[W, 128], [128 * W, 8], [1, W]])
                nc.sync.dma_start(
                    out_ap, onat[:, :].rearrange("p (s w) -> p s w", s=8))
                return onat

            # idx is a single buffer reused across tiles (bufs=1 semantics):
            # gather(i) reads it, so idxprep(i+1)'s write waits for gather(i)
            # (WAR) -- and gather(i+1) waits for the write (RAW).
            idx = scp.tile([128, S], I16, tag="idx")
            # guard scratch
            gsc = scp.tile([128, 1], F32, tag="gsc")

            # -------- warmup
            emit_dma_in(0)
            emit_dma_in(1)
            psts0 = emit_intrans_pe(0)
            xT0 = emit_intrans_act(0, psts0)
            ff_next = emit_idxprep(0, xT0, idx)

            # -------- main loop
            for i in range(n_tiles):
                ff = ff_next
                xT_prev = xT0
                sparse = emit_gather(i, idx)                  # phase A
                if i + 2 < n_tiles:
                    emit_dma_in(i + 2)                        # overlaps A
                psts = emit_intrans_pe(i + 1) if i + 1 < n_tiles else None

                # phase B (serial window, opens when gather i completes):
                outT = emit_extract_interp(i, sparse, ff, masks)
                onat = emit_out(i, outT)
                if i + 1 < n_tiles:
                    # guard 1: reads xT(i)'s buffer and outT(i).  Makes the
                    # ACT xT-copies for tile i+1 (writers of the xT buffer,
                    # WAR on this read) wait until tile i's interpolation is
                    # done -- so they cannot run during gather(i), where ACT
                    # activity stalls the GPSIMD SBUF port.
                    nc.vector.tensor_tensor(
                        gsc[:, :], xT_prev[:, 0:1], outT[:, 0:1], A.add)
                    xT = emit_intrans_act(i + 1, psts)
                    xT0 = xT
                    ff_next = emit_idxprep(i + 1, xT, idx)
                # guard 2: reads sparse(i) and onat(i).  Registers as a
                # reader of the sparse buffer, so gather(i+1) (its next
                # writer, WAR) waits until tile i's output copies are done.
                nc.vector.tensor_tensor(
                    gsc[:, :], sparse[:, 0:1], onat[:, 0:1], A.add)

    from concourse.library_overlay import lower_extended_insts
    lower_extended_insts(nc)
    _spill_waits(nc)
    return nc


def make_table(lookup_table):
    import ml_dtypes
    lut = np.asarray(lookup_table, dtype=np.float32)          # [4096, 64]
    vu = np.concatenate([lut[1:], lut[-1:]], axis=0)          # T[min(i+1,4095)]
    delta = vu - lut                                          # f32 exact
    val_bf = lut.astype(ml_dtypes.bfloat16)
    delta_bf = delta.astype(ml_dtypes.bfloat16)
    pair = np.stack([val_bf, delta_bf], axis=-1)              # [4096, 64, 2]
    tblw = np.ascontiguousarray(pair.transpose(1, 0, 2)).reshape(W, NB * 2)
    tbl128 = np.concatenate([tblw, tblw], axis=0)             # [128, 8192]
    return np.ascontiguousarray(tbl128)


def make_masks():
    p = np.arange(128)
    m = np.where(p[:, None] % 16 == np.arange(16)[None, :], -1, 0)
    return m.astype(np.int32)                                 # [128, 16]


_CACHE = {}


def kernel(inputs, lookup_table):
    x = np.ascontiguousarray(np.asarray(inputs, dtype=np.float32))
    B, T, Wx = x.shape
    assert Wx == W
    per_core_b = B // N_CORES
    n_rows = per_core_b * T
    tbl = make_table(lookup_table)
    ident = np.eye(128, dtype=np.float32)
    masks = make_masks()

    if n_rows not in _CACHE:
        _CACHE[n_rows] = build_kernel(n_rows)
    nc = _CACHE[n_rows]

    from concourse import bass_utils
    shards = x.reshape(N_CORES, n_rows, W)
    in_maps = [{"x": shards[c], "tbl": tbl, "ident": ident, "masks": masks}
               for c in range(N_CORES)]
    res = bass_utils.run_bass_kernel_spmd(
        nc, in_maps, core_ids=list(range(N_CORES)))
    out = np.stack([res.results[c]["y"] for c in range(N_CORES)], axis=0)
    return out.reshape(B, T, W)


# revision 16
# speedup vs baseline: 1.0737x; 1.0100x over previous
"""Trainium2 Bass kernel for nn_CachedShapingFunctions (embedding_lookup).

out[b,t,w] = interp of lookup_table[:, w] at uniform-bucket position of
inputs[b,t,w].  Data-parallel over batch across 8 NeuronCores; the LUT is
replicated as a host-prepared bf16 (value, delta) pair table per waveshaper.

Per-core pipeline (shard flattened to [65536, 64], 64 super-tiles of
[128 part = 2 time-chunks x 64 w, 512 t]):
  - DMA in natural tiles, PE-transpose to waveshaper-on-partition layout
  - DVE: 7-op index pipeline (clamp, round(ic-0.5) floor trick, fraction)
  - GPSIMD ap_gather of bf16 (value, delta) pairs (the dominant cost,
    ~372us/tile -- command-bound at ~45ns/idx)
  - strided-partition extraction of the 1/16-dense gather output
    (split ACT/DVE), interpolation, PE-transpose back, DMA out

Scheduling: ap_gather contends catastrophically with concurrent DVE work
(shared POOL/DVE SBUF port), so all DVE/ACT work is packed into a serial
window between gathers via buffer reuse (bufs=1 pools) and a dummy
dependency-chaining op.  DMA and PE overlap the gather freely.
"""
import sys
import numpy as np

sys.path.insert(0, '/opt/trn_rl_repo')

import bass_rust
import concourse.bass as bass
import concourse.mybir as mybir
import concourse.tile as tile_mod
from concourse.tile import TileContext
from concourse.vector_clock import ScopedClock
from concourse import library_config

MIN_VALUE, MAX_VALUE = -3.0, 3.0
NB = 4096          # buckets
W = 64             # waveshapers
N_CORES = 8

# ---------------------------------------------------------------- patches --
# This walrus build accepts at most ONE sync-wait per instruction.  The Tile
# tail drain and scheduler can attach more; spill the excess onto nops.

_MAXW = 1

def _spill_waits(nc):
    for f in nc.m.functions:
        for bb in f.blocks:
            out = []
            for inst in list(bb.instructions):
                si = inst.sync_info
                if si is not None and len(si.on_wait) > _MAXW:
                    waits = list(si.on_wait)
                    spill = waits[:-_MAXW]
                    for i in range(0, len(spill), _MAXW):
                        nop = mybir.InstNoOp(
                            name=f"wspill_{inst.name}_{i}", ins=[], outs=[])
                        nop.engine = inst.engine
                        nop.sync_info = bass_rust.SyncInfo(
                            on_wait=spill[i:i + _MAXW], on_update=[])
                        out.append(nop)
                    inst.sync_info = bass_rust.SyncInfo(
                        on_wait=waits[-_MAXW:], on_update=list(si.on_update))
                out.append(inst)
            bb.instructions = out


def _patched_drain_and_barrier(self, tick_clock, wait_clock):
    nc = self.nc
    drain_inst = nc.sync.drain()
    wait_clock.add_sem_waits(
        drain_inst.ins, ScopedClock({None: tick_clock.global_clock}))
    si = drain_inst.ins.sync_info
    if si is not None and len(si.on_wait) > _MAXW:
        waits = list(si.on_wait)
        drain_inst.ins.sync_info = bass_rust.SyncInfo(
            on_wait=waits[:_MAXW], on_update=list(si.on_update))
        rest = waits[_MAXW:]
        for i in range(0, len(rest), _MAXW):
            nop = nc.sync.nop(hint="drain_wait_spill", nofuse=True)
            nop.ins.sync_info = bass_rust.SyncInfo(
                on_wait=rest[i:i + _MAXW], on_update=[])
    nc.all_engine_barrier()
    assert self.sems is not None
    popped = nc._tile_sem_poison_stack.pop()
    assert popped is self._sem_poison
    nc.clear_and_free_semaphores(list(self.sems.allocated().values()))
    nc.all_engine_barrier()


tile_mod.TileContext._drain_and_barrier = _patched_drain_and_barrier

# ----------------------------------------------------------------- kernel --

S = 512            # t-columns per transposed super-tile (per chunk)
TROWS = 2 * S      # natural t rows covered per super-tile (2 chunks)

F32 = mybir.dt.float32
I16 = mybir.dt.int16
BF16 = mybir.dt.bfloat16


def build_kernel(n_rows):
    """n_rows: flattened time rows per core (65536 full scale)."""
    assert n_rows % TROWS == 0
    n_tiles = n_rows // TROWS
    nc = bass.Bass()
    x_d = nc.dram_tensor("x", [n_rows, W], F32, kind="ExternalInput")
    tbl_d = nc.dram_tensor("tbl", [128, NB * 2], BF16, kind="ExternalInput")
    id_d = nc.dram_tensor("ident", [128, 128], F32, kind="ExternalInput")
    mk_d = nc.dram_tensor("masks", [128, 16], mybir.dt.int32, kind="ExternalInput")
    y_d = nc.dram_tensor("y", [n_rows, W], F32, kind="ExternalOutput")

    A = mybir.AluOpType

    with TileContext(nc) as tc:
        with (
            tc.tile_pool(name="const", bufs=1) as cpool,
            tc.tile_pool(name="io", bufs=3) as iop,
            tc.tile_pool(name="xt", bufs=2) as xtp,
            tc.tile_pool(name="sc", bufs=2) as scp,
            tc.tile_pool(name="sp", bufs=2) as spp,
            tc.tile_pool(name="on", bufs=2) as onp,
            tc.tile_pool(name="psi", bufs=4, space="PSUM") as psip,
            tc.tile_pool(name="pso", bufs=2, space="PSUM") as psop,
        ):
            tbl = cpool.tile([128, NB * 2], BF16)
            ident = cpool.tile([128, 128], F32)
            masks = cpool.tile([128, 16], mybir.dt.int32)
            nc.sync.dma_start(tbl[:, :], tbl_d[:, :])
            nc.sync.dma_start(ident[:, :], id_d[:, :])
            nc.sync.dma_start(masks[:, :], mk_d[:, :])
            nc.gpsimd.load_library(library_config.ap_gather)
            tbl3 = tbl[:, :].rearrange("p (n d) -> p n d", d=2)

            xnats = {}

            def emit_dma_in(i):
                xnat = iop.tile([128, 8 * W], F32, tag="xnat")
                in_ap = bass.AP(x_d, i * TROWS * W, [[W, 128], [128 * W, 8], [1, W]])
                nc.sync.dma_start(
                    xnat[:, :].rearrange("p (s w) -> p s w", s=8), in_ap)
                xnats[i] = xnat

            def emit_intrans_pe(i):
                """PE transposes for tile i (runs free, overlaps gather)."""
                xnat = xnats.pop(i)
                psts = []
                for k in range(4):
                    pst = psip.tile([128, 128], F32, tag="psin")
                    nc.tensor.transpose(
                        pst[:, :], xnat[:, 128 * k: 128 * k + 128], ident)
                    psts.append(pst)
                return psts

            def emit_intrans_act(i, psts):
                """PSUM -> xT copies (ACT, in the serial window)."""
                xT = xtp.tile([128, S], F32, tag="xT")
                for k in range(4):
                    nc.scalar.copy(xT[:, 128 * k: 128 * k + 128], psts[k][:, :])
                return xT

            def emit_idxprep(i, xT, idx):
                """DVE index pipeline (7 ops): writes idx (i16) + fraction ff."""
                ic = scp.tile([128, S], F32, tag="ic")
                icc = scp.tile([128, S], F32, tag="icc")
                tmp = scp.tile([128, S], F32, tag="tmp")
                ilf = scp.tile([128, S], F32, tag="ilf")
                icm = scp.tile([128, S], F32, tag="icm")
                ff = scp.tile([128, S], F32, tag="ff")
                nc.vector.tensor_scalar(ic[:, :], xT[:, :], 3.0, 682.5, A.add, A.mult)
                nc.vector.tensor_scalar(icc[:, :], ic[:, :], 0.0, 4095.0, A.max, A.min)
                # floor(icc) via round-to-nearest(icc - (0.5 - eps)); off-by-one
                # at segment boundaries is harmless (interpolation continuity).
                nc.vector.tensor_scalar(tmp[:, :], icc[:, :], 0.49999997, None, A.subtract)
                nc.vector.tensor_copy(idx[:, :], tmp[:, :])          # f32 -> i16 RNE
                nc.vector.tensor_copy(ilf[:, :], idx[:, :])          # i16 -> f32
                # upper-clamped ic for the fraction: keeps lower extrapolation
                # exact and forces out = T[4095] for ic >= 4095.
                nc.vector.tensor_scalar(icm[:, :], ic[:, :], 4095.0, None, A.min)
                nc.vector.tensor_tensor(ff[:, :], icm[:, :], ilf[:, :], A.subtract)
                return ff

            def emit_gather(i, idx):
                sparse = spp.tile([128, 16 * S * 2], BF16, tag="sparse")
                sp3 = sparse[:, :].rearrange("p (n d) -> p n d", d=2)
                nc.gpsimd.ap_gather(sp3, tbl3, idx[:, :], channels=128,
                                    num_elems=NB, d=2, num_idxs=16 * S)
                return sparse

            def emit_extract_interp(i, sparse, ff, masks):
                """Compact the 1/16-dense gather output: partition p keeps
                pair slots m = 16n + (p%16).  Done as 16 bitwise masked
                merges in the int32 (pair = one u32) domain, then
                interpolate from the bf16 view."""
                pairs = scp.tile([128, S], mybir.dt.int32, tag="pairs")
                spi = sparse[:, :].bitcast(mybir.dt.int32)
                sp16 = spi.rearrange("p (n q) -> p n q", q=16)
                for r in range(16):
                    mcol = masks[:, r: r + 1]
                    if r == 0:
                        nc.vector.tensor_scalar(
                            pairs[:, :], sp16[:, :, 0], mcol, None, A.bitwise_and)
                    else:
                        nc.vector.scalar_tensor_tensor(
                            pairs[:, :], sp16[:, :, r], mcol, pairs[:, :],
                            A.bitwise_and, A.bitwise_or)
                pr3 = pairs[:, :].bitcast(BF16).rearrange("p (n q) -> p n q", q=2)
                outT = scp.tile([128, S], F32, tag="outT")
                nc.vector.tensor_tensor(outT[:, :], ff[:, :], pr3[:, :, 1], A.mult)
                nc.vector.tensor_tensor(outT[:, :], outT[:, :], pr3[:, :, 0], A.add)
                return outT

            def emit_out(i, outT):
                onat = onp.tile([128, 8 * W], F32, tag="onat")
                for k in range(4):
                    pst = psop.tile([128, 128], F32, tag="psout")
                    nc.tensor.transpose(
                        pst[:, :], outT[:, 128 * k: 128 * k + 128], ident)
                    nc.scalar.copy(onat[:, 128 * k: 128 * k + 128], pst[:, :])
                out_ap = bass.AP(y_d, i * TROWS * W, [[W, 128], [128 * W, 8], [1, W]])
                nc.sync.dma_start(
                    out_ap, onat[:, :].rearrange("p (s w) -> p s w", s=8))
                return onat

            # -------- warmup: prime two tiles of input + indices
            emit_dma_in(0)
            emit_dma_in(1)
            psts0 = emit_intrans_pe(0)
            xT0 = emit_intrans_act(0, psts0)
            idx0 = scp.tile([128, S], I16, tag="idx")
            ff0 = emit_idxprep(0, xT0, idx0)
            pend = (idx0, ff0)

            # -------- main loop: free-running pipeline (double buffers);
            # the gather paces everything, the rest overlaps it.
            for i in range(n_tiles):
                idx, ff = pend
                sparse = emit_gather(i, idx)
                if i + 2 < n_tiles:
                    emit_dma_in(i + 2)
                if i + 1 < n_tiles:
                    psts = emit_intrans_pe(i + 1)
                    xT = emit_intrans_act(i + 1, psts)
                    idxn = scp.tile([128, S], I16, tag="idx")
                    ffn = emit_idxprep(i + 1, xT, idxn)
                    pend = (idxn, ffn)
                outT = emit_extract_interp(i, sparse, ff, masks)
                emit_out(i, outT)

    from concourse.library_overlay import lower_extended_insts
    lower_extended_insts(nc)
    _spill_waits(nc)
    return nc


def make_table(lookup_table):
    import ml_dtypes
    lut = np.asarray(lookup_table, dtype=np.float32)          # [4096, 64]
    vu = np.concatenate([lut[1:], lut[-1:]], axis=0)          # T[min(i+1,4095)]
    delta = vu - lut                                          # f32 exact
    val_bf = lut.astype(ml_dtypes.bfloat16)
    delta_bf = delta.astype(ml_dtypes.bfloat16)
    pair = np.stack([val_bf, delta_bf], axis=-1)              # [4096, 64, 2]
    tblw = np.ascontiguousarray(pair.transpose(1, 0, 2)).reshape(W, NB * 2)
    tbl128 = np.concatenate([tblw, tblw], axis=0)             # [128, 8192]
    return np.ascontiguousarray(tbl128)


def make_masks():
    p = np.arange(128)
    m = np.where(p[:, None] % 16 == np.arange(16)[None, :], -1, 0)
    return m.astype(np.int32)                                 # [128, 16]


_CACHE = {}


def kernel(inputs, lookup_table):
    x = np.ascontiguousarray(np.asarray(inputs, dtype=np.float32))
    B, T, Wx = x.shape
    assert Wx == W
    per_core_b = B // N_CORES
    n_rows = per_core_b * T
    tbl = make_table(lookup_table)
    ident = np.eye(128, dtype=np.float32)
    masks = make_masks()

    if n_rows not in _CACHE:
        _CACHE[n_rows] = build_kernel(n_rows)
    nc = _CACHE[n_rows]

    from concourse import bass_utils
    shards = x.reshape(N_CORES, n_rows, W)
    in_maps = [{"x": shards[c], "tbl": tbl, "ident": ident, "masks": masks}
               for c in range(N_CORES)]
    res = bass_utils.run_bass_kernel_spmd(
        nc, in_maps, core_ids=list(range(N_CORES)))
    out = np.stack([res.results[c]["y"] for c in range(N_CORES)], axis=0)
    return out.reshape(B, T, W)
